# revision 9
# baseline (speedup 1.0000x reference)
"""ATLoss (adaptive-threshold multilabel loss) over 65536 length-8 segments.

Strategy: data-parallel over the 8 NeuronCores — core c takes segments
[c*8192, (c+1)*8192) plus the matching logits rows and labels rows.  Host
re-lays each core's slice out partition-major (segment -> SBUF partition) so
every DMA is large contiguous chunks per partition, then the device computes
per-segment
    loss = n_pos * log(x0 + sum(lab*x)) - sum(lab*e) + log(x0 + sum((1-lab)*x)) - th
with e = max over the 8 rows (cols 1..96), x = exp(e), x0 = exp(th),
accumulated as [128] per-partition partial sums.  Host sums 8x[128] partials
and divides by 65536 (the "all-reduce" of the scalar mean).

exp() needs no max-shift: inputs are standard-normal logits, |e| <= ~6, so
exp stays comfortably inside fp32 range (reference's max-shift is only for
numerical safety it does not need here either).
"""

import numpy as np

import concourse.bacc as bacc
import concourse.bass as bass
import concourse.mybir as mybir
import concourse.tile as tile
from concourse.bass_utils import run_bass_kernel_spmd

F32 = mybir.dt.float32
AX = mybir.AxisListType
ALU = mybir.AluOpType
ACTF = mybir.ActivationFunctionType

EP = 65536          # total segments (entity pairs)
L = 8               # rows per segment
C = 97              # classes (col 0 = threshold)
NCORES = 8
SEG_PER_CORE = EP // NCORES     # 8192
TILES = SEG_PER_CORE // 128     # 64 tiles of 128 segments
ROWF = L * C                    # 776 floats per segment
GROUP = 8                       # tiles per logits DMA (3.1 MB per transfer)
NGROUPS = TILES // GROUP


def _build_nc():
    nc = bacc.Bacc("TRN2", debug=False)
    logits_t = nc.dram_tensor("logits_t", [128, TILES * ROWF], F32, kind="ExternalInput")
    labels_t = nc.dram_tensor("labels_t", [128, TILES * C], F32, kind="ExternalInput")
    th_t = nc.dram_tensor("th_t", [128, TILES], F32, kind="ExternalInput")
    out = nc.dram_tensor("out", [128, 1], F32, kind="ExternalOutput")

    lg_view = logits_t.ap().rearrange("p (t f) -> p t f", f=ROWF)   # [128, 64, 776]
    lb_view = labels_t.ap().rearrange("p (t c) -> p t c", c=C)      # [128, 64, 97]

    with tile.TileContext(nc) as tc:
        with (
            tc.tile_pool(name="big", bufs=3) as big,
            tc.tile_pool(name="persist", bufs=1) as persist,
            tc.tile_pool(name="work", bufs=4) as work,
            tc.tile_pool(name="xe", bufs=64) as xe,
            tc.tile_pool(name="cols", bufs=1) as cols,
        ):
            lab_all = persist.tile([128, TILES, C], F32)
            nc.sync.dma_start(out=lab_all, in_=lb_view)
            th_sb = persist.tile([128, TILES], F32)
            nc.sync.dma_start(out=th_sb, in_=th_t.ap())

            SP1 = cols.tile([128, TILES], F32)   # sum(lab * exp(e))  cols 1..96
            DOT = cols.tile([128, TILES], F32)   # sum(lab * e)       cols 1..96
            S96 = cols.tile([128, TILES], F32)   # sum(exp(e))        cols 1..96
            NPOS = cols.tile([128, TILES], F32)  # sum(lab)           cols 1..96

            # n_pos for all tiles, batched (8 tiles per reduce)
            for g in range(NGROUPS):
                nc.vector.reduce_sum(
                    out=NPOS[:, g * GROUP:(g + 1) * GROUP],
                    in_=lab_all[:, g * GROUP:(g + 1) * GROUP, 1:],
                    axis=AX.X,
                )

            for g in range(NGROUPS):
                lg = big.tile([128, GROUP, ROWF], F32, tag="lg")
                nc.sync.dma_start(out=lg, in_=lg_view[:, g * GROUP:(g + 1) * GROUP, :])
                for j in range(GROUP):
                    t = g * GROUP + j
                    # [128, 776] -> [128 p, 97 c, 8 row]; cols 1..96 only
                    tv = lg[:, j, :].rearrange("p (l c) -> p c l", c=C)
                    e = xe.tile([128, C - 1], F32, tag="e")
                    nc.vector.reduce_max(out=e, in_=tv[:, 1:, :], axis=AX.X)
                    x = xe.tile([128, C - 1], F32, tag="x")
                    nc.scalar.activation(out=x, in_=e, func=ACTF.Exp)
                    scr1 = work.tile([128, C - 1], F32, tag="scr1")
                    nc.vector.scalar_tensor_tensor(
                        out=scr1,
                        in0=lab_all[:, t, 1:],
                        scalar=1.0,
                        in1=x,
                        op0=ALU.mult,
                        op1=ALU.mult,
                        accum_out=SP1[:, t:t + 1],
                    )
                    nc.vector.reduce_sum(
                        out=S96[:, t:t + 1], in_=x, axis=AX.X
                    )
                    scr2 = work.tile([128, C - 1], F32, tag="scr2")
                    nc.vector.scalar_tensor_tensor(
                        out=scr2,
                        in0=lab_all[:, t, 1:],
                        scalar=1.0,
                        in1=e,
                        op0=ALU.mult,
                        op1=ALU.mult,
                        accum_out=DOT[:, t:t + 1],
                    )

            # Final combine, batched over all 64 tile-columns.
            X0 = cols.tile([128, TILES], F32)
            nc.scalar.activation(out=X0, in_=th_sb, func=ACTF.Exp)
            SUMP = cols.tile([128, TILES], F32)
            nc.vector.tensor_add(SUMP, X0, SP1)
            LSEP = cols.tile([128, TILES], F32)
            nc.scalar.activation(out=LSEP, in_=SUMP, func=ACTF.Ln)
            TSUB = cols.tile([128, TILES], F32)
            nc.vector.tensor_sub(TSUB, S96, SP1)
            SN = cols.tile([128, TILES], F32)
            nc.vector.tensor_add(SN, TSUB, X0)
            LSEN = cols.tile([128, TILES], F32)
            nc.scalar.activation(out=LSEN, in_=SN, func=ACTF.Ln)

            T1 = cols.tile([128, TILES], F32)
            nc.vector.tensor_mul(T1, NPOS, LSEP)
            T2 = cols.tile([128, TILES], F32)
            nc.vector.tensor_sub(T2, T1, DOT)
            T3 = cols.tile([128, TILES], F32)
            nc.vector.tensor_add(T3, T2, LSEN)
            T4 = cols.tile([128, TILES], F32)
            nc.vector.tensor_sub(T4, T3, th_sb)

            partial = cols.tile([128, 1], F32)
            nc.vector.reduce_sum(out=partial, in_=T4, axis=AX.X)
            nc.sync.dma_start(out=out.ap(), in_=partial)
    # Bacc.compile legalizes multi-wait instructions into EventSemaphores,
    # inserts ACT table loads, and encodes InstISA bytes.
    nc.compile()
    return nc


_NC_CACHE = None


def _get_nc():
    global _NC_CACHE
    if _NC_CACHE is None:
        _NC_CACHE = _build_nc()
    return _NC_CACHE


def _shard_inputs(logits, labels):
    """Host-side shard + relayout: partition-major per core."""
    in_maps = []
    for c in range(NCORES):
        s0 = c * SEG_PER_CORE
        lg = (
            logits[s0 * L:(s0 + SEG_PER_CORE) * L]
            .reshape(TILES, 128, ROWF)
            .transpose(1, 0, 2)
            .reshape(128, TILES * ROWF)
        )
        lb = (
            labels[s0:s0 + SEG_PER_CORE]
            .reshape(TILES, 128, C)
            .transpose(1, 0, 2)
            .reshape(128, TILES * C)
        )
        th = logits[s0:s0 + SEG_PER_CORE, 0].reshape(TILES, 128).T
        in_maps.append(
            {
                "logits_t": np.ascontiguousarray(lg, dtype=np.float32),
                "labels_t": np.ascontiguousarray(lb, dtype=np.float32),
                "th_t": np.ascontiguousarray(th, dtype=np.float32),
            }
        )
    return in_maps


def _run(in_maps, trace=False, **kwargs):
    nc = _get_nc()
    return run_bass_kernel_spmd(nc, in_maps, list(range(NCORES)), trace=trace, **kwargs)


def _numpy_fallback(logits, labels, pos):
    """Generic reference math in numpy (any contiguous spans covering [0, N))."""
    ep = pos.shape[0]
    logits = logits.astype(np.float64)
    labels = labels.astype(np.float64)
    starts = pos[:, 0].astype(np.int64)
    e = np.maximum.reduceat(logits, starts, axis=0)
    e[:, 0] = logits[:ep, 0]
    lab = labels.copy()
    lab[:, 0] = 0.0
    x = np.exp(e)
    sp1 = (lab[:, 1:] * x[:, 1:]).sum(1)
    dot = (lab[:, 1:] * e[:, 1:]).sum(1)
    npos = lab[:, 1:].sum(1)
    s96 = x[:, 1:].sum(1)
    x0 = x[:, 0]
    lsep = np.log(x0 + sp1)
    lsen = np.log(x0 + s96 - sp1)
    loss = npos * lsep - dot + lsen - e[:, 0]
    return np.float32(loss.mean())


def kernel(logits, labels, pos):
    logits = np.ascontiguousarray(np.asarray(logits), dtype=np.float32)
    labels = np.ascontiguousarray(np.asarray(labels), dtype=np.float32)
    pos = np.asarray(pos)

    regular = (
        pos.shape == (EP, 2)
        and logits.shape == (EP * L, C)
        and int(pos[0, 0]) == 0
        and bool(np.all(pos[:, 1] - pos[:, 0] == L))
        and bool(np.all(pos[1:, 0] == pos[:-1, 1]))
    )
    if not regular:
        return _numpy_fallback(logits, labels, pos)

    in_maps = _shard_inputs(logits, labels)
    res = _run(in_maps)
    total = 0.0
    for r in res.results:
        total += float(r["out"].astype(np.float64).sum())
    return np.float32(total / EP)


# revision 10
# speedup vs baseline: 1.6168x; 1.6168x over previous
"""ATLoss (adaptive-threshold multilabel loss) over 65536 length-8 segments.

Strategy: data-parallel over the 8 NeuronCores — core c takes segments
[c*8192, (c+1)*8192) plus the matching logits rows and labels rows.  Host
re-lays each core's slice out partition-major (segment -> SBUF partition) so
every DMA is large contiguous chunks per partition; the device computes
per-segment
    loss = n_pos * log(x0 + sum(lab*x)) - sum(lab*e) + log(x0 + sum((1-lab)*x)) - th
with e = max over the 8 rows (cols 1..96), x = exp(e), x0 = exp(th),
accumulated as [128] per-partition partial sums.  Host sums 8x[128] partials
and divides by 65536 (the "all-reduce" of the scalar mean).

Logits/labels are cast f32 -> fp16 during the (SWDGE) DMA: the 8->1
segment max then runs in the DVE's 2x fp16 mode, halving the dominant
vector cost.  All reductions accumulate in fp32 (fp16 only quantizes the
max/exp inputs, ~5e-4 relative).  exp() needs no max-shift: inputs are
standard-normal logits, |e| <= ~6, well inside fp32/fp16 range.
"""

import numpy as np

import concourse.bacc as bacc
import concourse.bass as bass
import concourse.mybir as mybir
import concourse.tile as tile
from concourse.bass_utils import run_bass_kernel_spmd

F32 = mybir.dt.float32
F16 = mybir.dt.float16
AX = mybir.AxisListType
ALU = mybir.AluOpType
ACTF = mybir.ActivationFunctionType

EP = 65536          # total segments (entity pairs)
L = 8               # rows per segment
C = 97              # classes (col 0 = threshold)
NCORES = 8
SEG_PER_CORE = EP // NCORES     # 8192
TILES = SEG_PER_CORE // 128     # 64 tiles of 128 segments
ROWF = L * C                    # 776 floats per segment
GROUP = 8                       # tiles per logits DMA (3.1 MB f32 read per transfer)
NGROUPS = TILES // GROUP


def _build_nc():
    nc = bacc.Bacc("TRN2", debug=False)
    logits_t = nc.dram_tensor("logits_t", [128, TILES * ROWF], F32, kind="ExternalInput")
    labels_t = nc.dram_tensor("labels_t", [128, TILES * C], F32, kind="ExternalInput")
    th_t = nc.dram_tensor("th_t", [128, TILES], F32, kind="ExternalInput")
    out = nc.dram_tensor("out", [128, 1], F32, kind="ExternalOutput")

    lg_view = logits_t.ap().rearrange("p (t f) -> p t f", f=ROWF)   # [128, 64, 776]

    with tile.TileContext(nc) as tc:
        with (
            tc.tile_pool(name="big", bufs=3) as big,
            tc.tile_pool(name="persist", bufs=1) as persist,
            tc.tile_pool(name="work", bufs=4) as work,
            tc.tile_pool(name="xe", bufs=64) as xe,
            tc.tile_pool(name="cols", bufs=1) as cols,
        ):
            lab16 = persist.tile([128, TILES, C], F16)
            nc.gpsimd.dma_start(
                out=lab16, in_=labels_t.ap().rearrange("p (t c) -> p t c", c=C)
            )
            th_sb = persist.tile([128, TILES], F32)
            nc.sync.dma_start(out=th_sb, in_=th_t.ap())

            SP1 = cols.tile([128, TILES], F32)   # sum(lab * exp(e))  cols 1..96
            DOT = cols.tile([128, TILES], F32)   # sum(lab * e)       cols 1..96
            S96 = cols.tile([128, TILES], F32)   # sum(exp(e))        cols 1..96
            NPOS = cols.tile([128, TILES], F32)  # sum(lab)           cols 1..96

            for g in range(NGROUPS):
                lg16 = big.tile([128, GROUP, ROWF], F16, tag="lg16")
                # SWDGE cast DMA: reads f32 from HBM, lands fp16 in SBUF
                nc.gpsimd.dma_start(
                    out=lg16, in_=lg_view[:, g * GROUP:(g + 1) * GROUP, :]
                )
                for j in range(GROUP):
                    t = g * GROUP + j
                    row = lg16[:, j, :]
                    # fp16 max tree (stride-1, 2x DVE mode): 776 -> 388 -> 194 -> 97
                    m1 = work.tile([128, 388], F16, tag="m1")
                    nc.vector.tensor_max(m1, row[:, 0:388], row[:, 388:776])
                    m2 = work.tile([128, 194], F16, tag="m2")
                    nc.vector.tensor_max(m2, m1[:, 0:194], m1[:, 194:388])
                    e16 = xe.tile([128, C], F16, tag="e16")
                    nc.vector.tensor_max(e16, m2[:, 0:97], m2[:, 97:194])
                    x16 = xe.tile([128, C - 1], F16, tag="x16")
                    nc.scalar.activation(
                        out=x16, in_=e16[:, 1:], func=ACTF.Exp,
                        accum_out=S96[:, t:t + 1],
                    )
                    scr1 = work.tile([128, C - 1], F16, tag="scr1")
                    nc.vector.scalar_tensor_tensor(
                        out=scr1, in0=lab16[:, t, 1:], scalar=1.0, in1=x16,
                        op0=ALU.mult, op1=ALU.mult, accum_out=SP1[:, t:t + 1],
                    )
                    scr2 = work.tile([128, C - 1], F16, tag="scr2")
                    nc.vector.scalar_tensor_tensor(
                        out=scr2, in0=lab16[:, t, 1:], scalar=1.0, in1=e16[:, 1:],
                        op0=ALU.mult, op1=ALU.mult, accum_out=DOT[:, t:t + 1],
                    )
                    scr3 = work.tile([128, C - 1], F16, tag="scr3")
                    nc.scalar.activation(
                        out=scr3, in_=lab16[:, t, 1:], func=ACTF.Copy,
                        accum_out=NPOS[:, t:t + 1],
                    )

            # Final combine, batched over all 64 tile-columns (fp32).
            X0 = cols.tile([128, TILES], F32)
            nc.scalar.activation(out=X0, in_=th_sb, func=ACTF.Exp)
            SUMP = cols.tile([128, TILES], F32)
            nc.vector.tensor_add(SUMP, X0, SP1)
            LSEP = cols.tile([128, TILES], F32)
            nc.scalar.activation(out=LSEP, in_=SUMP, func=ACTF.Ln)
            TSUB = cols.tile([128, TILES], F32)
            nc.vector.tensor_sub(TSUB, S96, SP1)
            SN = cols.tile([128, TILES], F32)
            nc.vector.tensor_add(SN, TSUB, X0)
            LSEN = cols.tile([128, TILES], F32)
            nc.scalar.activation(out=LSEN, in_=SN, func=ACTF.Ln)

            T1 = cols.tile([128, TILES], F32)
            nc.vector.tensor_mul(T1, NPOS, LSEP)
            T2 = cols.tile([128, TILES], F32)
            nc.vector.tensor_sub(T2, T1, DOT)
            T3 = cols.tile([128, TILES], F32)
            nc.vector.tensor_add(T3, T2, LSEN)
            T4 = cols.tile([128, TILES], F32)
            nc.vector.tensor_sub(T4, T3, th_sb)

            partial = cols.tile([128, 1], F32)
            nc.vector.reduce_sum(out=partial, in_=T4, axis=AX.X)
            nc.sync.dma_start(out=out.ap(), in_=partial)
    # Bacc.compile legalizes multi-wait instructions into EventSemaphores,
    # inserts ACT table loads, and encodes InstISA bytes.
    nc.compile()
    return nc


_NC_CACHE = None


def _get_nc():
    global _NC_CACHE
    if _NC_CACHE is None:
        _NC_CACHE = _build_nc()
    return _NC_CACHE


def _shard_inputs(logits, labels):
    """Host-side shard + relayout: partition-major per core."""
    in_maps = []
    for c in range(NCORES):
        s0 = c * SEG_PER_CORE
        lg = (
            logits[s0 * L:(s0 + SEG_PER_CORE) * L]
            .reshape(TILES, 128, ROWF)
            .transpose(1, 0, 2)
            .reshape(128, TILES * ROWF)
        )
        lb = (
            labels[s0:s0 + SEG_PER_CORE]
            .reshape(TILES, 128, C)
            .transpose(1, 0, 2)
            .reshape(128, TILES * C)
        )
        th = logits[s0:s0 + SEG_PER_CORE, 0].reshape(TILES, 128).T
        in_maps.append(
            {
                "logits_t": np.ascontiguousarray(lg, dtype=np.float32),
                "labels_t": np.ascontiguousarray(lb, dtype=np.float32),
                "th_t": np.ascontiguousarray(th, dtype=np.float32),
            }
        )
    return in_maps


def _run(in_maps, trace=False, **kwargs):
    nc = _get_nc()
    return run_bass_kernel_spmd(nc, in_maps, list(range(NCORES)), trace=trace, **kwargs)


def _numpy_fallback(logits, labels, pos):
    """Generic reference math in numpy (any contiguous spans covering [0, N))."""
    ep = pos.shape[0]
    logits = logits.astype(np.float64)
    labels = labels.astype(np.float64)
    starts = pos[:, 0].astype(np.int64)
    e = np.maximum.reduceat(logits, starts, axis=0)
    e[:, 0] = logits[:ep, 0]
    lab = labels.copy()
    lab[:, 0] = 0.0
    x = np.exp(e)
    sp1 = (lab[:, 1:] * x[:, 1:]).sum(1)
    dot = (lab[:, 1:] * e[:, 1:]).sum(1)
    npos = lab[:, 1:].sum(1)
    s96 = x[:, 1:].sum(1)
    x0 = x[:, 0]
    lsep = np.log(x0 + sp1)
    lsen = np.log(x0 + s96 - sp1)
    loss = npos * lsep - dot + lsen - e[:, 0]
    return np.float32(loss.mean())


def kernel(logits, labels, pos):
    logits = np.ascontiguousarray(np.asarray(logits), dtype=np.float32)
    labels = np.ascontiguousarray(np.asarray(labels), dtype=np.float32)
    pos = np.asarray(pos)

    regular = (
        pos.shape == (EP, 2)
        and logits.shape == (EP * L, C)
        and int(pos[0, 0]) == 0
        and bool(np.all(pos[:, 1] - pos[:, 0] == L))
        and bool(np.all(pos[1:, 0] == pos[:-1, 1]))
    )
    if not regular:
        return _numpy_fallback(logits, labels, pos)

    in_maps = _shard_inputs(logits, labels)
    res = _run(in_maps)
    total = 0.0
    for r in res.results:
        total += float(r["out"].astype(np.float64).sum())
    return np.float32(total / EP)
